# revision 2
# baseline (speedup 1.0000x reference)
"""HBMP (3-branch LSTM + BiLSTM + global max pool) Trainium2 kernel.

Model (B=64, T=512, E=300, H=512, NB=3 branches):
  per branch: h1 = LSTM(x); hf = LSTM(h1); hb = rev(LSTM(rev(h1)))
  emb = maxpool_T(concat([hf, hb], -1));  out = concat over branches [B, 3*2H]

Mapping onto 6 NeuronCores (task-parallel; batch stays whole because the
recurrent matmul cost is weight-streaming-bound, independent of batch):
  core c in 0..5 handles (branch = c%3, direction = fwd if c<3 else bwd):
    P0: xz_u = x @ Wx_u + b_u                  (dense matmul, M-tiled)
    P1: uni LSTM scan -> transposed h stream hT to DRAM
    P2: xz_d = h1 @ Wx_d + b_d                 (dense matmul over hT tiles;
        written T-REVERSED via indirect-DMA scatter for bwd cores, so one
        SPMD program serves both directions - direction lives in the
        per-core scatter-index table input)
    P3: dir LSTM scan over xz_d with running max -> rmax [64, 512]
Host gathers the 6 rmax outputs into [64, 3072].

Scan step: z (PSUM, [64, 4H]) accumulates xz_t (via identity matmul) plus
h_{t-1} @ Wh (4 K-tile matmuls with the transposed state hT as stationary);
gates on ScalarE from PSUM; c/h updates on VectorE; h re-transposed on PE.
"""
import sys

sys.path.insert(0, "/opt/trn_rl_repo")

import numpy as np

B, T, E, H = 64, 512, 300, 512
FOUR_H = 4 * H
NB = 3

_CACHE = {}


def _build_program(rep=1):
    import concourse.bass as bass
    import concourse.tile as tile
    from concourse import bacc, mybir

    F32 = mybir.dt.float32
    I32 = mybir.dt.int32
    Sig = mybir.ActivationFunctionType.Sigmoid
    Tanh = mybir.ActivationFunctionType.Tanh

    nc = bacc.Bacc("TRN2", target_bir_lowering=False, debug=False,
                   enable_asserts=False, num_devices=6)

    d = {}
    d["xTu"] = nc.dram_tensor("xTu", [T, 128, 3, 64], F32, kind="ExternalInput").ap()
    d["wxu"] = nc.dram_tensor("wxu", [128, 3, FOUR_H], F32, kind="ExternalInput").ap()
    d["whu"] = nc.dram_tensor("whu", [128, 4, FOUR_H], F32, kind="ExternalInput").ap()
    d["bu"] = nc.dram_tensor("bu", [128, FOUR_H], F32, kind="ExternalInput").ap()
    d["wxd"] = nc.dram_tensor("wxd", [128, 4, FOUR_H], F32, kind="ExternalInput").ap()
    d["whd"] = nc.dram_tensor("whd", [128, 4, FOUR_H], F32, kind="ExternalInput").ap()
    d["bd"] = nc.dram_tensor("bd", [128, FOUR_H], F32, kind="ExternalInput").ap()
    d["id64"] = nc.dram_tensor("id64", [64, 64], F32, kind="ExternalInput").ap()
    d["scat"] = nc.dram_tensor("scat", [128, T // 2], I32, kind="ExternalInput").ap()
    d["xzu"] = nc.dram_tensor("xzu", [T, B, FOUR_H], F32, kind="Internal").ap()
    d["hT"] = nc.dram_tensor("hT", [T, 128, 256], F32, kind="Internal").ap()
    d["xzd"] = nc.dram_tensor("xzd", [T, B, FOUR_H], F32, kind="Internal").ap()
    d["rmax"] = nc.dram_tensor("rmax", [B, H], F32, kind="ExternalOutput").ap()

    def build_xproj(tc):
        with (
            tc.tile_pool(name="p0w", bufs=1) as wp,
            tc.tile_pool(name="p0io", bufs=4) as iop,
            tc.tile_pool(name="p0ps", bufs=2, space="PSUM") as psp,
        ):
            wx_sb = wp.tile([128, 3, FOUR_H], F32, tag="wx")
            nc.sync.dma_start(wx_sb[:], d["wxu"])
            b_sb = wp.tile([128, FOUR_H], F32, tag="b")
            nc.sync.dma_start(b_sb[:], d["bu"])
            for m in range(T // 2):
                xt = iop.tile([128, 3, 2, 64], F32, tag="xt")
                nc.sync.dma_start(
                    xt[:], d["xTu"][2 * m:2 * m + 2].rearrange("t p k b -> p k t b"))
                zp = psp.tile([128, FOUR_H], F32, tag="zp")
                for k in range(3):
                    for n in range(4):
                        nc.tensor.matmul(
                            zp[:, bass.ts(n, 512)],
                            xt[:, k, :, :].rearrange("p t b -> p (t b)"),
                            wx_sb[:, k, bass.ts(n, 512)],
                            start=(k == 0), stop=(k == 2))
                zs = iop.tile([128, FOUR_H], F32, tag="zs")
                nc.vector.tensor_add(zs[:], zp[:], b_sb[:])
                nc.sync.dma_start(
                    d["xzu"][2 * m:2 * m + 2].rearrange("t b n -> (t b) n"), zs[:])

    def build_scan(tc, xz, wh_name, store_hT, rmax_out):
        # gate column order is host-permuted to [f i o g]:
        #   chunks: n0=f, n1=i, n2=o, n3=g
        with (
            tc.tile_pool(name=f"w_{wh_name}", bufs=1) as whp,
            tc.tile_pool(name=f"st_{wh_name}", bufs=1) as statep,
            tc.tile_pool(name=f"xz_{wh_name}", bufs=4) as xzp,
            tc.tile_pool(name=f"g_{wh_name}", bufs=2) as gp,
            tc.tile_pool(name=f"zps_{wh_name}", bufs=1, space="PSUM") as zpsp,
            tc.tile_pool(name=f"tps_{wh_name}", bufs=2, space="PSUM") as tpsp,
        ):
            wh_sb = whp.tile([128, 4, FOUR_H], F32, tag="wh")
            nc.sync.dma_start(wh_sb[:], d[wh_name])
            id_sb = whp.tile([64, 64], F32, tag="id")
            nc.sync.dma_start(id_sb[:], d["id64"])

            hT_sb = statep.tile([128, 4, 64], F32, tag="hT")
            # st = [c | tanh(g)] adjacent so one DVE mul makes [f*c | i*tg]
            st_sb = statep.tile([64, 2 * H], F32, tag="st")
            nc.vector.memset(hT_sb[:], 0.0)
            nc.vector.memset(st_sb[:], 0.0)
            if rmax_out is not None:
                rmax_sb = statep.tile([64, H], F32, tag="rmax")
                nc.vector.memset(rmax_sb[:], -1e30)

            for t in range(T):
                xz_t = xzp.tile([64, FOUR_H], F32, tag="xzt")
                nc.sync.dma_start(xz_t[:], xz[t])
                z = zpsp.tile([64, FOUR_H], F32, tag="z")
                for k in range(4):
                    for n in range(4):
                        nc.tensor.matmul(z[:, bass.ts(n, 512)], hT_sb[:, k, :],
                                         wh_sb[:, k, bass.ts(n, 512)],
                                         start=(k == 0), stop=(k == 3))
                zf = gp.tile([64, FOUR_H], F32, tag="zf")
                nc.vector.tensor_add(zf[:], z[:], xz_t[:])
                ga = gp.tile([64, 3 * H], F32, tag="ga")  # [sf si so]
                nc.scalar.activation(ga[:], zf[:, 0:3 * H], Sig)
                nc.scalar.activation(st_sb[:, H:2 * H], zf[:, 3 * H:4 * H], Tanh)
                t12 = gp.tile([64, 2 * H], F32, tag="t12")
                nc.vector.tensor_mul(t12[:], ga[:, 0:2 * H], st_sb[:])
                nc.vector.tensor_add(st_sb[:, 0:H], t12[:, 0:H], t12[:, H:2 * H])
                tc_t = gp.tile([64, H], F32, tag="tc")
                nc.scalar.activation(tc_t[:], st_sb[:, 0:H], Tanh)
                h_t = gp.tile([64, H], F32, tag="h")
                nc.vector.tensor_mul(h_t[:], ga[:, 2 * H:3 * H], tc_t[:])
                if rmax_out is not None:
                    nc.vector.tensor_max(rmax_sb[:], rmax_sb[:], h_t[:])
                pT = tpsp.tile([128, 4, 64], F32, tag="pT")
                for k in range(4):
                    nc.tensor.transpose(pT[:, k, :], h_t[:, bass.ts(k, 128)], id_sb[:])
                nc.vector.tensor_copy(hT_sb[:], pT[:])
                if store_hT:
                    nc.sync.dma_start(d["hT"][t],
                                      hT_sb[:].rearrange("p k b -> p (k b)"))
            if rmax_out is not None:
                nc.sync.dma_start(rmax_out, rmax_sb[:])

    def build_hproj(tc):
        with (
            tc.tile_pool(name="p2w", bufs=1) as wp,
            tc.tile_pool(name="p2io", bufs=4) as iop,
            tc.tile_pool(name="p2ps", bufs=2, space="PSUM") as psp,
        ):
            wx_sb = wp.tile([128, 4, FOUR_H], F32, tag="wx")
            nc.sync.dma_start(wx_sb[:], d["wxd"])
            b_sb = wp.tile([128, FOUR_H], F32, tag="b")
            nc.sync.dma_start(b_sb[:], d["bd"])
            scat_sb = wp.tile([128, T // 2], I32, tag="scat")
            nc.sync.dma_start(scat_sb[:], d["scat"])
            xzd_rows = d["xzd"].rearrange("t b n -> (t b) n")
            for m in range(T // 2):
                ht = iop.tile([128, 4, 2, 64], F32, tag="ht")
                nc.sync.dma_start(
                    ht[:],
                    d["hT"][2 * m:2 * m + 2].rearrange("t p (k b) -> p k t b", k=4))
                zp = psp.tile([128, FOUR_H], F32, tag="zp")
                for k in range(4):
                    for n in range(4):
                        nc.tensor.matmul(
                            zp[:, bass.ts(n, 512)],
                            ht[:, k, :, :].rearrange("p t b -> p (t b)"),
                            wx_sb[:, k, bass.ts(n, 512)],
                            start=(k == 0), stop=(k == 3))
                zs = iop.tile([128, FOUR_H], F32, tag="zs")
                nc.vector.tensor_add(zs[:], zp[:], b_sb[:])
                nc.gpsimd.indirect_dma_start(
                    out=xzd_rows,
                    out_offset=bass.IndirectOffsetOnAxis(
                        ap=scat_sb[:, m:m + 1], axis=0),
                    in_=zs[:],
                    in_offset=None)

    with tile.TileContext(nc) as tc:
        for _ in range(rep):
            build_xproj(tc)
            build_scan(tc, d["xzu"], "whu", store_hT=True, rmax_out=None)
            build_hproj(tc)
            build_scan(tc, d["xzd"], "whd", store_hT=False, rmax_out=d["rmax"])
    nc.compile()
    return nc


def _prep_shared(x):
    """x [B,T,E] -> xT [T,128,3,64] with xT[t,p,k,b] = x[b,t,k*128+p] (E pad 384)."""
    xpad = np.zeros((B, T, 384), np.float32)
    xpad[:, :, :E] = x
    xT = xpad.transpose(1, 2, 0).reshape(T, 3, 128, B).transpose(0, 2, 1, 3)
    return np.ascontiguousarray(xT)


_GATE_PERM = np.r_[H:2 * H, 0:H, 3 * H:4 * H, 2 * H:3 * H]  # [i f g o]->[f i o g]


def _prep_core(xT, wx_u, wh_u, b_u, wx_d, wh_d, b_d, reverse):
    wx_u = np.asarray(wx_u, np.float32)[:, _GATE_PERM]
    wh_u = np.asarray(wh_u, np.float32)[:, _GATE_PERM]
    b_u = np.asarray(b_u, np.float32)[_GATE_PERM]
    wx_d = np.asarray(wx_d, np.float32)[:, _GATE_PERM]
    wh_d = np.asarray(wh_d, np.float32)[:, _GATE_PERM]
    b_d = np.asarray(b_d, np.float32)[_GATE_PERM]
    wxu_pad = np.zeros((384, FOUR_H), np.float32)
    wxu_pad[:E] = wx_u
    p = np.arange(128)
    m = np.arange(T // 2)
    t_src = 2 * m[None, :] + (p[:, None] >= 64)
    t_dst = (T - 1 - t_src) if reverse else t_src
    scat = (t_dst * 64 + (p[:, None] % 64)).astype(np.int32)
    return {
        "xTu": xT,
        "wxu": np.ascontiguousarray(
            wxu_pad.reshape(3, 128, FOUR_H).transpose(1, 0, 2)),
        "whu": np.ascontiguousarray(
            np.asarray(wh_u, np.float32).reshape(4, 128, FOUR_H).transpose(1, 0, 2)),
        "bu": np.ascontiguousarray(
            np.broadcast_to(np.asarray(b_u, np.float32), (128, FOUR_H))),
        "wxd": np.ascontiguousarray(
            np.asarray(wx_d, np.float32).reshape(4, 128, FOUR_H).transpose(1, 0, 2)),
        "whd": np.ascontiguousarray(
            np.asarray(wh_d, np.float32).reshape(4, 128, FOUR_H).transpose(1, 0, 2)),
        "bd": np.ascontiguousarray(
            np.broadcast_to(np.asarray(b_d, np.float32), (128, FOUR_H))),
        "id64": np.eye(64, dtype=np.float32),
        "scat": scat,
    }


class _Runner:
    """Compile the bass program to a PJRT executable ONCE and reuse it.

    run_bass_kernel_spmd builds a fresh jax.jit per call, which re-loads the
    NEFF onto the devices every call (~seconds for a 37k-instruction program,
    scaling with program size). That load time is host/runtime overhead, not
    device execution, so we memoize the jitted callable and keep the big
    inputs device-resident for the timing loop.
    """

    def __init__(self, nc, n_cores):
        import jax
        from jax.sharding import Mesh, PartitionSpec
        from jax.experimental.shard_map import shard_map
        from concourse import bass2jax, mybir

        bass2jax.install_neuronx_cc_hook()
        assert nc.dbg_addr is None or not nc.dbg_callbacks
        self._jax = jax
        self._n_cores = n_cores
        partition_name = (nc.partition_id_tensor.name
                          if nc.partition_id_tensor else None)
        in_names, out_names, out_avals = [], [], []
        for alloc in nc.m.functions[0].allocations:
            if not isinstance(alloc, mybir.MemoryLocationSet):
                continue
            name = alloc.memorylocations[0].name
            if alloc.kind == "ExternalInput":
                if name != partition_name and name != getattr(
                        getattr(nc, "dbg_addr", None), "name", None):
                    in_names.append(name)
            elif alloc.kind == "ExternalOutput":
                out_avals.append(jax.core.ShapedArray(
                    tuple(alloc.tensor_shape), mybir.dt.np(alloc.dtype)))
                out_names.append(name)
        self._in_names, self._out_names = in_names, out_names
        self._out_avals = out_avals
        n_params, n_outs = len(in_names), len(out_avals)
        bind_in_names = list(in_names) + list(out_names)
        extra_zero_inputs = []
        if nc.dbg_addr is not None:
            bind_in_names.append(nc.dbg_addr.name)
            extra_zero_inputs.append(np.zeros((1, 2), np.uint32))
        if partition_name is not None:
            bind_in_names.append(partition_name)
        self._extra_zero_inputs = extra_zero_inputs

        def _body(*args):
            operands = list(args)
            if partition_name is not None:
                operands.append(bass2jax.partition_id_tensor())
            outs = bass2jax._bass_exec_p.bind(
                *operands,
                out_avals=tuple(out_avals),
                in_names=tuple(bind_in_names),
                out_names=tuple(out_names),
                lowering_input_output_aliases=(),
                sim_require_finite=True,
                sim_require_nnan=True,
                nc=nc,
            )
            return tuple(outs)

        devices = jax.devices()[:n_cores]
        assert len(devices) == n_cores
        mesh = Mesh(np.asarray(devices), ("core",))
        n_extra = len(extra_zero_inputs)
        in_specs = (PartitionSpec("core"),) * (n_params + n_outs + n_extra)
        out_specs = (PartitionSpec("core"),) * n_outs
        self._fn = jax.jit(
            shard_map(_body, mesh=mesh, in_specs=in_specs,
                      out_specs=out_specs, check_rep=False),
            keep_unused=True,
        )
        self._dev_inputs = None
        self._dev_inputs_key = None

    def upload(self, in_maps):
        """Concatenate per-core inputs and park them on the devices."""
        import jax
        n = self._n_cores
        per_core = [[np.asarray(m[name]) for name in self._in_names]
                    for m in in_maps]
        concat_in = [
            np.concatenate([per_core[c][i] for c in range(n)], axis=0)
            for i in range(len(self._in_names))
        ]
        concat_zeros = [
            np.zeros((n * a.shape[0], *a.shape[1:]), a.dtype)
            for a in self._out_avals
        ]
        concat_extra = [
            np.concatenate([z] * n, axis=0) for z in self._extra_zero_inputs
        ]
        args = concat_in + concat_zeros + concat_extra
        self._dev_inputs = [jax.device_put(a) for a in args]
        jax.block_until_ready(self._dev_inputs)

    def execute(self):
        out = self._fn(*self._dev_inputs)
        self._jax.block_until_ready(out)
        return out

    def run(self, in_maps):
        self.upload(in_maps)
        out_arrs = self.execute()
        n = self._n_cores
        return [
            {name: np.asarray(out_arrs[i]).reshape(
                n, *self._out_avals[i].shape)[c]
             for i, name in enumerate(self._out_names)}
            for c in range(n)
        ]


class _Result:
    def __init__(self, results):
        self.results = results


def _get_runner(rep=1):
    key = f"runner{rep}"
    if key not in _CACHE:
        _CACHE[key] = _Runner(_build_program(rep), 6)
    return _CACHE[key]


def _run(in_maps, rep=1):
    return _Result(_get_runner(rep).run(in_maps))


def build_in_maps(x, uni_Wx, uni_Wh, uni_b, fwd_Wx, fwd_Wh, fwd_b,
                  bwd_Wx, bwd_Wh, bwd_b):
    xT = _prep_shared(np.asarray(x, np.float32))
    in_maps = []
    for c in range(6):
        br = c % 3
        if c < 3:
            wx_d, wh_d, b_d, rev = fwd_Wx[br], fwd_Wh[br], fwd_b[br], False
        else:
            wx_d, wh_d, b_d, rev = bwd_Wx[br], bwd_Wh[br], bwd_b[br], True
        in_maps.append(_prep_core(xT, np.asarray(uni_Wx[br], np.float32),
                                  uni_Wh[br], uni_b[br], wx_d, wh_d, b_d, rev))
    return in_maps


def kernel(x, uni_Wx, uni_Wh, uni_b, fwd_Wx, fwd_Wh, fwd_b,
           bwd_Wx, bwd_Wh, bwd_b):
    in_maps = build_in_maps(x, uni_Wx, uni_Wh, uni_b, fwd_Wx, fwd_Wh, fwd_b,
                            bwd_Wx, bwd_Wh, bwd_b)
    res = _run(in_maps)
    out = np.empty((B, NB * 2 * H), np.float32)
    for c in range(6):
        br = c % 3
        off = br * 2 * H + (0 if c < 3 else H)
        out[:, off:off + H] = res.results[c]["rmax"]
    return out



# revision 9
# speedup vs baseline: 1.9559x; 1.9559x over previous
"""HBMP (3-branch LSTM + BiLSTM + global max pool) Trainium2 kernel.

Model (B=64, T=512, E=300, H=512, NB=3 branches):
  per branch: h1 = LSTM(x); hf = LSTM(h1); hb = rev(LSTM(rev(h1)))
  emb = maxpool_T(concat([hf, hb], -1));  out = concat over branches [B, 3*2H]

Mapping onto 6 NeuronCores (task-parallel; batch stays whole because the
recurrent matmul cost is weight-streaming-bound, independent of batch):
  core c in 0..5 handles (branch = c%3, direction = fwd if c<3 else bwd):
    P0: xz_u = x @ Wx_u + b_u                  (dense matmul, M-tiled)
    P1: uni LSTM scan -> transposed h stream hT to DRAM
    P2: xz_d = h1 @ Wx_d + b_d                 (dense matmul over hT tiles;
        written T-REVERSED via indirect-DMA scatter for bwd cores, so one
        SPMD program serves both directions - direction lives in the
        per-core scatter-index table input)
    P3: dir LSTM scan over xz_d with running max -> rmax [64, 512]
Host gathers the 6 rmax outputs into [64, 3072].

Scan step: z (PSUM, [64, 4H]) accumulates xz_t (via identity matmul) plus
h_{t-1} @ Wh (4 K-tile matmuls with the transposed state hT as stationary);
gates on ScalarE from PSUM; c/h updates on VectorE; h re-transposed on PE.
"""
import sys

sys.path.insert(0, "/opt/trn_rl_repo")

import numpy as np

B, T, E, H = 64, 512, 300, 512
FOUR_H = 4 * H
NB = 3

_CACHE = {}


def _build_program(rep=1):
    import concourse.bass as bass
    import concourse.tile as tile
    from concourse import bacc, mybir

    F32 = mybir.dt.float32
    I32 = mybir.dt.int32
    Sig = mybir.ActivationFunctionType.Sigmoid
    Tanh = mybir.ActivationFunctionType.Tanh

    nc = bacc.Bacc("TRN2", target_bir_lowering=False, debug=False,
                   enable_asserts=False, num_devices=6)

    d = {}
    d["xTu"] = nc.dram_tensor("xTu", [T // 2, 128, 3, 2, 64], F32,
                              kind="ExternalInput").ap()
    d["wxu"] = nc.dram_tensor("wxu", [128, 3, FOUR_H], F32, kind="ExternalInput").ap()
    d["whu"] = nc.dram_tensor("whu", [128, 4, FOUR_H], F32, kind="ExternalInput").ap()
    d["bu"] = nc.dram_tensor("bu", [128, FOUR_H], F32, kind="ExternalInput").ap()
    d["wxd"] = nc.dram_tensor("wxd", [128, 4, FOUR_H], F32, kind="ExternalInput").ap()
    d["whd"] = nc.dram_tensor("whd", [128, 4, FOUR_H], F32, kind="ExternalInput").ap()
    d["bd"] = nc.dram_tensor("bd", [128, FOUR_H], F32, kind="ExternalInput").ap()
    d["id64"] = nc.dram_tensor("id64", [64, 64], F32, kind="ExternalInput").ap()
    d["scat"] = nc.dram_tensor("scat", [128, T // 2], I32, kind="ExternalInput").ap()
    d["xzu"] = nc.dram_tensor("xzu", [T, B, FOUR_H], F32, kind="Internal").ap()
    d["hT"] = nc.dram_tensor("hT", [T, 128, 256], F32, kind="Internal").ap()
    d["xzd"] = nc.dram_tensor("xzd", [T, B, FOUR_H], F32, kind="Internal").ap()
    d["rmax"] = nc.dram_tensor("rmax", [B, H], F32, kind="ExternalOutput").ap()

    def build_xproj(tc):
        with (
            tc.tile_pool(name="p0w", bufs=1) as wp,
            tc.tile_pool(name="p0io", bufs=4) as iop,
            tc.tile_pool(name="p0ps", bufs=2, space="PSUM") as psp,
        ):
            wx_sb = wp.tile([128, 3, FOUR_H], F32, tag="wx")
            nc.sync.dma_start(wx_sb[:], d["wxu"])
            b_sb = wp.tile([128, FOUR_H], F32, tag="b")
            nc.sync.dma_start(b_sb[:], d["bu"])
            for m in range(T // 2):
                xt = iop.tile([128, 3, 2, 64], F32, tag="xt")
                nc.sync.dma_start(xt[:], d["xTu"][m])
                zp = psp.tile([128, FOUR_H], F32, tag="zp")
                for k in range(3):
                    for n in range(4):
                        nc.tensor.matmul(
                            zp[:, bass.ts(n, 512)],
                            xt[:, k, :, :].rearrange("p t b -> p (t b)"),
                            wx_sb[:, k, bass.ts(n, 512)],
                            start=(k == 0), stop=(k == 2))
                zs = iop.tile([128, FOUR_H], F32, tag="zs")
                nc.vector.tensor_add(zs[:], zp[:], b_sb[:])
                nc.sync.dma_start(
                    d["xzu"][2 * m:2 * m + 2].rearrange("t b n -> (t b) n"), zs[:])

    def build_scan(tc, xz, wh_name, store_hT, rmax_out):
        # gate column order is host-permuted to [g f i o]:
        #   PSUM banks: z0=g, z1=f, z2=i, z3=o.  xz_t is accumulated into
        #   PSUM by an identity matmul (K=64) instead of a DVE add, so the
        #   gate activations read PSUM directly.  g first so the
        #   tanh(g) -> c -> tanh(c) chain overlaps the f/i/o matmuls and
        #   only sigmoid(o) -> h -> transpose trails the last matmul.
        with (
            tc.tile_pool(name=f"w_{wh_name}", bufs=1) as whp,
            tc.tile_pool(name=f"st_{wh_name}", bufs=1) as statep,
            tc.tile_pool(name=f"xz_{wh_name}", bufs=6) as xzp,
            tc.tile_pool(name=f"g_{wh_name}", bufs=2) as gp,
            tc.tile_pool(name=f"zps_{wh_name}", bufs=1, space="PSUM") as zpsp,
            tc.tile_pool(name=f"tps_{wh_name}", bufs=2, space="PSUM") as tpsp,
        ):
            wh_sb = whp.tile([128, 4, FOUR_H], F32, tag="wh")
            nc.sync.dma_start(wh_sb[:], d[wh_name])
            id_sb = whp.tile([64, 64], F32, tag="id")
            nc.sync.dma_start(id_sb[:], d["id64"])

            hT_sb = statep.tile([128, 4, 64], F32, tag="hT")
            # st = [c | tanh(g)] adjacent so one DVE mul makes [f*c | i*tg]
            st_sb = statep.tile([64, 2, H], F32, tag="st")
            nc.vector.memset(hT_sb[:], 0.0)
            nc.vector.memset(st_sb[:], 0.0)
            if rmax_out is not None:
                rmax_sb = statep.tile([64, H], F32, tag="rmax")
                nc.vector.memset(rmax_sb[:], -1e30)

            for t in range(T):
                xz_t = xzp.tile([64, FOUR_H], F32, tag="xzt")
                nc.sync.dma_start(xz_t[:], xz[t])
                zn = []
                for n in range(4):
                    z = zpsp.tile([64, 512], F32, tag=f"z{n}")
                    for k in range(4):
                        nc.tensor.matmul(z[:], hT_sb[:, k, :],
                                         wh_sb[:, k, bass.ts(n, 512)],
                                         start=(k == 0), stop=False)
                    nc.tensor.matmul(z[:], id_sb[:], xz_t[:, bass.ts(n, 512)],
                                     start=False, stop=True)
                    zn.append(z)
                ga = gp.tile([64, 3, H], F32, tag="ga")  # [sf si so]
                t12 = gp.tile([64, 2, H], F32, tag="t12")
                tc_t = gp.tile([64, H], F32, tag="tc")
                h_t = gp.tile([64, H], F32, tag="h")
                pT = tpsp.tile([128, 4, 64], F32, tag="pT")
                # tg as soon as bank g lands; sf/si behind banks f/i
                for c2 in range(2):
                    nc.scalar.activation(st_sb[:, 1, bass.ts(c2, 256)],
                                         zn[0][:, bass.ts(c2, 256)], Tanh)
                nc.scalar.activation(ga[:, 0, :], zn[1][:], Sig)
                nc.scalar.activation(ga[:, 1, :], zn[2][:], Sig)
                # c update overlaps bank o's matmuls
                for c2 in range(2):
                    sl = bass.ts(c2, 256)
                    nc.vector.tensor_mul(t12[:, :, sl], ga[:, 0:2, sl],
                                         st_sb[:, :, sl])
                    nc.vector.tensor_add(st_sb[:, 0, sl], t12[:, 0, sl],
                                         t12[:, 1, sl])
                    nc.scalar.activation(tc_t[:, sl], st_sb[:, 0, sl], Tanh)
                # only this trails the last matmul: so -> h -> transpose
                for c2 in range(2):
                    sl = bass.ts(c2, 256)
                    nc.scalar.activation(ga[:, 2, sl], zn[3][:, sl], Sig)
                    nc.vector.tensor_mul(h_t[:, sl], ga[:, 2, sl], tc_t[:, sl])
                    for j in range(2):
                        jj = 2 * c2 + j
                        nc.tensor.transpose(pT[:, jj, :],
                                            h_t[:, bass.ts(jj, 128)], id_sb[:])
                    nc.vector.tensor_copy(hT_sb[:, 2 * c2:2 * c2 + 2, :],
                                          pT[:, 2 * c2:2 * c2 + 2, :])
                if rmax_out is not None:
                    nc.vector.tensor_max(rmax_sb[:], rmax_sb[:], h_t[:])
                if store_hT:
                    nc.sync.dma_start(d["hT"][t],
                                      hT_sb[:].rearrange("p k b -> p (k b)"))
            if rmax_out is not None:
                nc.sync.dma_start(rmax_out, rmax_sb[:])

    def build_hproj(tc):
        with (
            tc.tile_pool(name="p2w", bufs=1) as wp,
            tc.tile_pool(name="p2io", bufs=4) as iop,
            tc.tile_pool(name="p2ps", bufs=2, space="PSUM") as psp,
        ):
            wx_sb = wp.tile([128, 4, FOUR_H], F32, tag="wx")
            nc.sync.dma_start(wx_sb[:], d["wxd"])
            b_sb = wp.tile([128, FOUR_H], F32, tag="b")
            nc.sync.dma_start(b_sb[:], d["bd"])
            scat_sb = wp.tile([128, T // 2], I32, tag="scat")
            nc.sync.dma_start(scat_sb[:], d["scat"])
            xzd_rows = d["xzd"].rearrange("t b n -> (t b) n")
            for m in range(T // 2):
                ht = iop.tile([128, 4, 2, 64], F32, tag="ht")
                nc.sync.dma_start(
                    ht[:],
                    d["hT"][2 * m:2 * m + 2].rearrange("t p (k b) -> p k t b", k=4))
                zp = psp.tile([128, FOUR_H], F32, tag="zp")
                for k in range(4):
                    for n in range(4):
                        nc.tensor.matmul(
                            zp[:, bass.ts(n, 512)],
                            ht[:, k, :, :].rearrange("p t b -> p (t b)"),
                            wx_sb[:, k, bass.ts(n, 512)],
                            start=(k == 0), stop=(k == 3))
                zs = iop.tile([128, FOUR_H], F32, tag="zs")
                nc.vector.tensor_add(zs[:], zp[:], b_sb[:])
                nc.gpsimd.indirect_dma_start(
                    out=xzd_rows,
                    out_offset=bass.IndirectOffsetOnAxis(
                        ap=scat_sb[:, m:m + 1], axis=0),
                    in_=zs[:],
                    in_offset=None)

    with tile.TileContext(nc) as tc:
        for _ in range(rep):
            build_xproj(tc)
            build_scan(tc, d["xzu"], "whu", store_hT=True, rmax_out=None)
            build_hproj(tc)
            build_scan(tc, d["xzd"], "whd", store_hT=False, rmax_out=d["rmax"])
    nc.compile()
    return nc


def _prep_shared(x):
    """x [B,T,E] -> xT [T/2,128,3,2,64]: xT[m,p,k,t,b] = x[b,2m+t,k*128+p]
    (E padded to 384); pair-major so the P0 tile load is one contiguous DMA."""
    xpad = np.zeros((B, T, 384), np.float32)
    xpad[:, :, :E] = x
    xT = xpad.transpose(1, 2, 0).reshape(T, 3, 128, B).transpose(0, 2, 1, 3)
    xT = xT.reshape(T // 2, 2, 128, 3, 64).transpose(0, 2, 3, 1, 4)
    return np.ascontiguousarray(xT)


_GATE_PERM = np.r_[2 * H:3 * H, H:2 * H, 0:H, 3 * H:4 * H]  # [i f g o]->[g f i o]


def _prep_core(xT, wx_u, wh_u, b_u, wx_d, wh_d, b_d, reverse):
    wx_u = np.asarray(wx_u, np.float32)[:, _GATE_PERM]
    wh_u = np.asarray(wh_u, np.float32)[:, _GATE_PERM]
    b_u = np.asarray(b_u, np.float32)[_GATE_PERM]
    wx_d = np.asarray(wx_d, np.float32)[:, _GATE_PERM]
    wh_d = np.asarray(wh_d, np.float32)[:, _GATE_PERM]
    b_d = np.asarray(b_d, np.float32)[_GATE_PERM]
    wxu_pad = np.zeros((384, FOUR_H), np.float32)
    wxu_pad[:E] = wx_u
    p = np.arange(128)
    m = np.arange(T // 2)
    t_src = 2 * m[None, :] + (p[:, None] >= 64)
    t_dst = (T - 1 - t_src) if reverse else t_src
    scat = (t_dst * 64 + (p[:, None] % 64)).astype(np.int32)
    return {
        "xTu": xT,
        "wxu": np.ascontiguousarray(
            wxu_pad.reshape(3, 128, FOUR_H).transpose(1, 0, 2)),
        "whu": np.ascontiguousarray(
            np.asarray(wh_u, np.float32).reshape(4, 128, FOUR_H).transpose(1, 0, 2)),
        "bu": np.ascontiguousarray(
            np.broadcast_to(np.asarray(b_u, np.float32), (128, FOUR_H))),
        "wxd": np.ascontiguousarray(
            np.asarray(wx_d, np.float32).reshape(4, 128, FOUR_H).transpose(1, 0, 2)),
        "whd": np.ascontiguousarray(
            np.asarray(wh_d, np.float32).reshape(4, 128, FOUR_H).transpose(1, 0, 2)),
        "bd": np.ascontiguousarray(
            np.broadcast_to(np.asarray(b_d, np.float32), (128, FOUR_H))),
        "id64": np.eye(64, dtype=np.float32),
        "scat": scat,
    }


class _Runner:
    """Compile the bass program to a PJRT executable ONCE and reuse it.

    run_bass_kernel_spmd builds a fresh jax.jit per call, which re-loads the
    NEFF onto the devices every call (~seconds for a 37k-instruction program,
    scaling with program size). That load time is host/runtime overhead, not
    device execution, so we memoize the jitted callable and keep the big
    inputs device-resident for the timing loop.
    """

    def __init__(self, nc, n_cores):
        import jax
        from jax.sharding import Mesh, PartitionSpec
        from jax.experimental.shard_map import shard_map
        from concourse import bass2jax, mybir

        bass2jax.install_neuronx_cc_hook()
        assert nc.dbg_addr is None or not nc.dbg_callbacks
        self._jax = jax
        self._n_cores = n_cores
        partition_name = (nc.partition_id_tensor.name
                          if nc.partition_id_tensor else None)
        in_names, out_names, out_avals = [], [], []
        for alloc in nc.m.functions[0].allocations:
            if not isinstance(alloc, mybir.MemoryLocationSet):
                continue
            name = alloc.memorylocations[0].name
            if alloc.kind == "ExternalInput":
                if name != partition_name and name != getattr(
                        getattr(nc, "dbg_addr", None), "name", None):
                    in_names.append(name)
            elif alloc.kind == "ExternalOutput":
                out_avals.append(jax.core.ShapedArray(
                    tuple(alloc.tensor_shape), mybir.dt.np(alloc.dtype)))
                out_names.append(name)
        self._in_names, self._out_names = in_names, out_names
        self._out_avals = out_avals
        n_params, n_outs = len(in_names), len(out_avals)
        bind_in_names = list(in_names) + list(out_names)
        extra_zero_inputs = []
        if nc.dbg_addr is not None:
            bind_in_names.append(nc.dbg_addr.name)
            extra_zero_inputs.append(np.zeros((1, 2), np.uint32))
        if partition_name is not None:
            bind_in_names.append(partition_name)
        self._extra_zero_inputs = extra_zero_inputs

        def _body(*args):
            operands = list(args)
            if partition_name is not None:
                operands.append(bass2jax.partition_id_tensor())
            outs = bass2jax._bass_exec_p.bind(
                *operands,
                out_avals=tuple(out_avals),
                in_names=tuple(bind_in_names),
                out_names=tuple(out_names),
                lowering_input_output_aliases=(),
                sim_require_finite=True,
                sim_require_nnan=True,
                nc=nc,
            )
            return tuple(outs)

        devices = jax.devices()[:n_cores]
        assert len(devices) == n_cores
        mesh = Mesh(np.asarray(devices), ("core",))
        n_extra = len(extra_zero_inputs)
        in_specs = (PartitionSpec("core"),) * (n_params + n_outs + n_extra)
        out_specs = (PartitionSpec("core"),) * n_outs
        self._fn = jax.jit(
            shard_map(_body, mesh=mesh, in_specs=in_specs,
                      out_specs=out_specs, check_rep=False),
            keep_unused=True,
        )
        self._dev_inputs = None
        self._dev_inputs_key = None

    def upload(self, in_maps):
        """Concatenate per-core inputs and park them on the devices."""
        import jax
        n = self._n_cores
        per_core = [[np.asarray(m[name]) for name in self._in_names]
                    for m in in_maps]
        concat_in = [
            np.concatenate([per_core[c][i] for c in range(n)], axis=0)
            for i in range(len(self._in_names))
        ]
        concat_zeros = [
            np.zeros((n * a.shape[0], *a.shape[1:]), a.dtype)
            for a in self._out_avals
        ]
        concat_extra = [
            np.concatenate([z] * n, axis=0) for z in self._extra_zero_inputs
        ]
        args = concat_in + concat_zeros + concat_extra
        self._dev_inputs = [jax.device_put(a) for a in args]
        jax.block_until_ready(self._dev_inputs)

    def execute(self):
        out = self._fn(*self._dev_inputs)
        self._jax.block_until_ready(out)
        return out

    def run(self, in_maps):
        self.upload(in_maps)
        out_arrs = self.execute()
        n = self._n_cores
        return [
            {name: np.asarray(out_arrs[i]).reshape(
                n, *self._out_avals[i].shape)[c]
             for i, name in enumerate(self._out_names)}
            for c in range(n)
        ]


class _Result:
    def __init__(self, results):
        self.results = results


def _get_runner(rep=1):
    key = f"runner{rep}"
    if key not in _CACHE:
        _CACHE[key] = _Runner(_build_program(rep), 6)
    return _CACHE[key]


def _run(in_maps, rep=1):
    return _Result(_get_runner(rep).run(in_maps))


def build_in_maps(x, uni_Wx, uni_Wh, uni_b, fwd_Wx, fwd_Wh, fwd_b,
                  bwd_Wx, bwd_Wh, bwd_b):
    xT = _prep_shared(np.asarray(x, np.float32))
    in_maps = []
    for c in range(6):
        br = c % 3
        if c < 3:
            wx_d, wh_d, b_d, rev = fwd_Wx[br], fwd_Wh[br], fwd_b[br], False
        else:
            wx_d, wh_d, b_d, rev = bwd_Wx[br], bwd_Wh[br], bwd_b[br], True
        in_maps.append(_prep_core(xT, np.asarray(uni_Wx[br], np.float32),
                                  uni_Wh[br], uni_b[br], wx_d, wh_d, b_d, rev))
    return in_maps


def kernel(x, uni_Wx, uni_Wh, uni_b, fwd_Wx, fwd_Wh, fwd_b,
           bwd_Wx, bwd_Wh, bwd_b):
    in_maps = build_in_maps(x, uni_Wx, uni_Wh, uni_b, fwd_Wx, fwd_Wh, fwd_b,
                            bwd_Wx, bwd_Wh, bwd_b)
    res = _run(in_maps)
    out = np.empty((B, NB * 2 * H), np.float32)
    for c in range(6):
        br = c % 3
        off = br * 2 * H + (0 if c < 3 else H)
        out[:, off:off + H] = res.results[c]["rmax"]
    return out

